# revision 25
# baseline (speedup 1.0000x reference)
import sys

import numpy as np

try:
    import concourse.bass as bass
except ImportError:
    sys.path.insert(0, "/opt/trn_rl_repo")
    import concourse.bass as bass

import concourse.bacc as bacc
import concourse.mybir as mybir
import concourse.tile as tile
from concourse.bass_utils import run_bass_kernel_spmd

F32 = mybir.dt.float32
BF16 = mybir.dt.bfloat16
B, S, D = 2, 2048, 1024
NH, DH = 16, 64
HPC = 4            # heads per core
HF = HPC * DH      # 256 per-core head features
TQ = 512           # attention i-chunk (query) size
NJT = S // 128     # 16 key tiles
QB = 128           # per-core output rows per 512-query block
SCALE = 1.0 / float(np.sqrt(DH))

_CACHE = {}


def _build_graph(variant="full"):
    sim = variant == "sim"
    nc = bacc.Bacc(num_devices=8)

    xqT = nc.dram_tensor("xqT", [D, S], BF16, kind="ExternalInput")
    xkT = nc.dram_tensor("xkT", [D, S], BF16, kind="ExternalInput")
    xvT = nc.dram_tensor("xvT", [D, S], BF16, kind="ExternalInput")
    wqT = nc.dram_tensor("wqT", [D, HF], BF16, kind="ExternalInput")
    wkT = nc.dram_tensor("wkT", [D, HF], BF16, kind="ExternalInput")
    wvT = nc.dram_tensor("wvT", [D, HF], BF16, kind="ExternalInput")
    woT = nc.dram_tensor("woT", [D, D], BF16, kind="ExternalInput")
    # lower-triangular ones (incl. diagonal) in [key, query] orientation
    tri = nc.dram_tensor("tri", [128, 128], BF16, kind="ExternalInput")
    eye = nc.dram_tensor("eye", [128, 128], BF16, kind="ExternalInput")
    # sel[0, r] == 1 iff this core's group rank is r (predicates the
    # post-AllGather staging DMA for this core's query blocks)
    sel = nc.dram_tensor("sel", [1, 4], mybir.dt.uint32, kind="ExternalInput")
    out_q = nc.dram_tensor("out_q", [4 * QB, D], F32, kind="ExternalOutput")

    Exp = mybir.ActivationFunctionType.Exp

    with tile.TileContext(nc) as tc:
        sel_regs = []
        for j in range(4):
            r = nc.sync.alloc_register(f"selreg{j}")
            nc.sync.reg_load(r, sel[0:1, j:j + 1])
            sel_regs.append(nc.sync.snap(r, donate=True, min_val=0, max_val=1))
        with (
            tc.tile_pool(name="dram", bufs=1, space="DRAM") as dramp,
            tc.tile_pool(name="const", bufs=1) as constp,
            tc.tile_pool(name="persist", bufs=1) as pers,
            tc.tile_pool(name="weights", bufs=1) as wpool,
            tc.tile_pool(name="xstage", bufs=1) as xpool,
            tc.tile_pool(name="attn", bufs=6) as apool,
            tc.tile_pool(name="ctx", bufs=2) as cpool,
            tc.tile_pool(name="rb", bufs=2) as rbpool,
            tc.tile_pool(name="rv", bufs=2) as rvpool,
            tc.tile_pool(name="cst", bufs=4) as cstp,
            tc.tile_pool(name="obuf", bufs=2) as obp,
            tc.tile_pool(name="ps_a", bufs=3, space="PSUM") as ps_a,
            tc.tile_pool(name="ps_c", bufs=2, space="PSUM") as ps_c,
        ):
            # collective staging: ccin = this core's ctx features for the
            # whole sequence; agout = concat over the 4-core group.
            ccin = dramp.tile([HF, S], BF16, name="ccin")
            agout = dramp.tile([4 * HF, S], BF16, name="agout")

            # DMA emission order matters: the first projection matmul needs
            # wq + the first xq slice, so load those first and defer the big
            # wo / const loads until after the x streams are queued.
            wq_sb = wpool.tile([128, 8, HF], BF16, name="wq_sb")
            wk_sb = wpool.tile([128, 8, HF], BF16, name="wk_sb")
            wv_sb = wpool.tile([128, 8, HF], BF16, name="wv_sb")
            xq_sb = xpool.tile([128, 8, S], BF16, name="xq_sb")
            xk_sb = xpool.tile([128, 8, S], BF16, name="xk_sb")
            xv_sb = xpool.tile([128, 8, S], BF16, name="xv_sb")
            for w_sb, wsrc, xs, xsrc in (
                (wq_sb, wqT, xq_sb, xqT),
                (wk_sb, wkT, xk_sb, xkT),
                (wv_sb, wvT, xv_sb, xvT),
            ):
                for kd in range(8):
                    nc.sync.dma_start(
                        w_sb[:, kd, :], wsrc[bass.ts(kd, 128), :]
                    )
                    nc.sync.dma_start(xs[:, kd, :], xsrc[bass.ts(kd, 128), :])

            tri_sb = constp.tile([128, 128], BF16, name="tri_sb")
            nc.sync.dma_start(tri_sb[:], tri[:, :])
            eye_sb = constp.tile([128, 128], BF16, name="eye_sb")
            nc.sync.dma_start(eye_sb[:], eye[:, :])
            wo_sb = wpool.tile([128, 8, D], BF16, name="wo_sb")
            nc.sync.dma_start(wo_sb[:], woT[:, :].rearrange("(n p) d -> p n d", p=128))

            # Persistent Q^T/K^T/V^T in bf16: tile u holds heads (2u, 2u+1)
            # stacked on partitions (64 each). Vb is V in natural orientation
            # with a ones column (row 64 of the AV product = softmax
            # denominator).
            QT = [pers.tile([128, S], BF16, name=f"QT{u}") for u in range(2)]
            KT = [pers.tile([128, S], BF16, name=f"KT{u}") for u in range(2)]
            VT = [pers.tile([128, S], BF16, name=f"VT{u}") for u in range(2)]
            Vb = pers.tile([128, NJT * HPC, DH + 1], BF16, name="Vb")
            nc.vector.memset(Vb[:, :, DH], 1.0)

            # Projections: stationary weight block, stream the sequence in
            # 2x512-col pieces per [128, 2, 512] psum tile (psum bank limit
            # is 512 f32 per matmul output).
            def proj_stream(xs, w_sb, dsts):
                for u in range(2):
                    for half in range(2):
                        ps = ps_a.tile([128, 2, TQ], F32, name="psa")
                        for kd in range(8):
                            for c in range(2):
                                nc.tensor.matmul(
                                    ps[:, c, :],
                                    w_sb[:, kd, bass.ts(u, 128)],
                                    xs[:, kd, half * 1024 + c * TQ:
                                       half * 1024 + (c + 1) * TQ],
                                    start=(kd == 0),
                                    stop=(kd == 7),
                                )
                        nc.vector.tensor_copy(
                            dsts[u][:, bass.ts(half, 1024)],
                            ps[:].rearrange("p c t -> p (c t)"),
                        )

            proj_stream(xq_sb, wq_sb, QT)
            proj_stream(xk_sb, wk_sb, KT)
            proj_stream(xv_sb, wv_sb, VT)

            # V transpose: [feat, t] -> natural [t, feat] blocks of Vb via
            # PE transpose (stationary = the block, identity streams).
            Vb4 = Vb[:, :, 0:DH].rearrange("p (j f) c -> p j f c", f=HPC)
            for u in range(2):
                for g in range(2):
                    pst = ps_a.tile([128, 8, 128], BF16, name="psa")
                    for j in range(8):
                        nc.tensor.transpose(
                            pst[:, j, :],
                            VT[u][:, bass.ts(g * 8 + j, 128)],
                            eye_sb[:],
                        )
                    nc.vector.tensor_copy(
                        Vb4[:, g * 8:g * 8 + 8, 2 * u:2 * u + 2, :],
                        pst[:].rearrange("p j (h c) -> p j h c", h=2),
                    )

            # Attention for i-chunk ic, head pairs u=(2u, 2u+1).  Scores for
            # both heads of a pair go into one 2-bank PSUM tile, one merged
            # exp per j-tile.  Causality handled by free-dim trimming on the
            # diagonal chunk + a 0/1 triangle multiply on the diagonal
            # 128-block (no -1e9 mask add needed).
            skew = 3
            for ic in range(4):
                n_jt = 4 * ic + 4
                for u in range(2):
                    pctx = [
                        ps_c.tile([DH + 1, TQ], F32, name="psc") for _ in range(2)
                    ]
                    ats = []
                    lows = []
                    for jt in range(n_jt):
                        p = jt - 4 * ic
                        lo = max(p, 0) * 128
                        ps2 = ps_a.tile([128, 2, TQ], F32, name="psa")
                        for h in range(2):
                            nc.tensor.matmul(
                                ps2[:, h, lo:TQ],
                                KT[u][h * DH:(h + 1) * DH, bass.ts(jt, 128)],
                                QT[u][h * DH:(h + 1) * DH, ic * TQ + lo:(ic + 1) * TQ],
                                start=True,
                                stop=True,
                            )
                        at2 = apool.tile([128, 2, TQ], BF16, name="at2")
                        nc.scalar.activation(
                            at2[:, :, lo:TQ], ps2[:, :, lo:TQ], Exp, scale=SCALE
                        )
                        if p >= 0:
                            for h in range(2):
                                nc.vector.tensor_mul(
                                    at2[:, h, lo:lo + 128],
                                    at2[:, h, lo:lo + 128],
                                    tri_sb[:],
                                )
                        ats.append(at2)
                        lows.append(lo)
                        # AV accumulation skewed behind scores for PE/ACT
                        # pipelining
                        if jt >= skew:
                            pv = jt - skew
                            for h in range(2):
                                nc.tensor.matmul(
                                    pctx[h][:, lows[pv]:TQ],
                                    Vb[:, pv * HPC + 2 * u + h, :],
                                    ats[pv][:, h, lows[pv]:TQ],
                                    start=(pv == 0),
                                    stop=False,
                                )
                    # Drain the AV tail per head and normalize immediately:
                    # h=0's psum slot frees while h=1's tail matmuls run.
                    # Row DH of pctx is the softmax denominator; 1/denom is
                    # broadcast across the 64 feature partitions on Pool.
                    for h in range(2):
                        gh = 2 * u + h
                        for pv in range(max(n_jt - skew, 0), n_jt):
                            nc.tensor.matmul(
                                pctx[h][:, lows[pv]:TQ],
                                Vb[:, pv * HPC + gh, :],
                                ats[pv][:, h, lows[pv]:TQ],
                                start=(pv == 0),
                                stop=(pv == n_jt - 1),
                            )
                        rv = rvpool.tile([1, TQ], F32, name="rvec")
                        nc.vector.reciprocal(rv[:], pctx[h][DH:DH + 1, :])
                        rb = rbpool.tile([DH, TQ], F32, name="rbt")
                        nc.gpsimd.partition_broadcast(rb[:], rv[:])
                        ctxT = cpool.tile([DH, TQ], BF16, name="ctxT")
                        nc.vector.tensor_mul(ctxT[:], pctx[h][0:DH, :], rb[:])
                        nc.sync.dma_start(
                            ccin[gh * DH:(gh + 1) * DH, bass.ts(ic, TQ)],
                            ctxT[:],
                        )

            if sim:
                for r in range(4):
                    nc.sync.dma_start(
                        agout[r * HF:(r + 1) * HF, :], ccin[:, :]
                    )
            else:
                nc.gpsimd.collective_compute(
                    "AllGather",
                    mybir.AluOpType.bypass,
                    replica_groups=[[0, 1, 2, 3], [4, 5, 6, 7]],
                    ins=[ccin.opt()],
                    outs=[agout.opt()],
                )

            # Output projection: this core's 128-query block of each 512
            # chunk. Prefetch all four staged [1024 feats, 128 q] slices for
            # group rank r via predicated DMAs (exactly one per chunk fires
            # at runtime), then stream Wo against stationary ctx blocks.
            csts = []
            for ic in range(4):
                cst = cstp.tile([128, 8, QB], BF16, name="cst")
                if sim:
                    nc.sync.dma_start(
                        cst[:],
                        agout[:, ic * TQ:ic * TQ + QB].rearrange(
                            "(n p) q -> p n q", p=128
                        ),
                    )
                else:
                    for r in range(4):
                        nc.sync.dma_start(
                            cst[:],
                            agout[:, ic * TQ + r * QB:ic * TQ + (r + 1) * QB]
                            .rearrange("(n p) q -> p n q", p=128),
                            cond=sel_regs[r],
                        )
                csts.append(cst)
            for ic in range(4):
                pso = ps_a.tile([128, 2, TQ], F32, name="psa")
                for kt in range(8):
                    for c in range(2):
                        nc.tensor.matmul(
                            pso[0:QB, c, :],
                            csts[ic][:, kt, :],
                            wo_sb[:, kt, bass.ts(c, TQ)],
                            start=(kt == 0),
                            stop=(kt == 7),
                        )
                ob = obp.tile([QB, D], F32, name="ob")
                # alternate evacuation engines so psum slots free fast
                if ic % 2 == 0:
                    nc.vector.tensor_copy(
                        ob[:], pso[0:QB, :].rearrange("p c t -> p (c t)")
                    )
                else:
                    nc.scalar.copy(
                        ob[:], pso[0:QB, :].rearrange("p c t -> p (c t)")
                    )
                nc.sync.dma_start(out_q[bass.ts(ic, QB), :], ob[:])

    nc.finalize()
    return nc


def _make_in_maps(inputs):
    import ml_dtypes

    bf16 = ml_dtypes.bfloat16
    query, key, value = inputs["query"], inputs["key"], inputs["value"]
    Wq, Wk, Wv, Wo = inputs["Wq"], inputs["Wk"], inputs["Wv"], inputs["Wo"]

    tri_blk = np.tril(np.ones((128, 128), np.float32)).T.astype(bf16)
    eye_blk = np.eye(128, dtype=np.float32).astype(bf16)
    woT_full = np.ascontiguousarray(np.asarray(Wo, np.float32).T).astype(bf16)

    xT = {}
    for b in range(2):
        xT[("q", b)] = np.ascontiguousarray(np.asarray(query[b], np.float32).T).astype(bf16)
        xT[("k", b)] = np.ascontiguousarray(np.asarray(key[b], np.float32).T).astype(bf16)
        xT[("v", b)] = np.ascontiguousarray(np.asarray(value[b], np.float32).T).astype(bf16)

    in_maps = []
    for c in range(8):
        b, r = divmod(c, 4)
        rs = slice(r * HF, (r + 1) * HF)
        in_maps.append(
            {
                "xqT": xT[("q", b)],
                "xkT": xT[("k", b)],
                "xvT": xT[("v", b)],
                "wqT": np.ascontiguousarray(np.asarray(Wq[rs], np.float32).T).astype(bf16),
                "wkT": np.ascontiguousarray(np.asarray(Wk[rs], np.float32).T).astype(bf16),
                "wvT": np.ascontiguousarray(np.asarray(Wv[rs], np.float32).T).astype(bf16),
                "woT": woT_full,
                "tri": tri_blk,
                "eye": eye_blk,
                "sel": (np.arange(4, dtype=np.uint32) == r).astype(np.uint32)[None, :],
            }
        )
    return in_maps


def _run(inputs, trace=False):
    if "nc" not in _CACHE:
        _CACHE["nc"] = _build_graph()
    nc = _CACHE["nc"]
    in_maps = _make_in_maps(inputs)
    res = run_bass_kernel_spmd(nc, in_maps, core_ids=list(range(8)), trace=trace)

    out = np.empty((B, S, D), np.float32)
    for c in range(8):
        b, r = divmod(c, 4)
        oq = np.asarray(res.results[c]["out_q"])
        for ic in range(4):
            out[b, ic * TQ + r * QB:ic * TQ + (r + 1) * QB, :] = oq[
                ic * QB:(ic + 1) * QB, :
            ]
    return out, res


def kernel(**inputs):
    out, _ = _run(inputs, trace=False)
    return out
